# revision 31
# baseline (speedup 1.0000x reference)
"""AttentionWithRoPE Trainium2 kernel (8-core SPMD), all-bf16 PE pipeline.

Sharding: core c handles batch b = c // 2 and head-group g = c % 2
(heads 4g..4g+3).  Each core computes rmsnorm(x_b), its 4 heads' Q/K/V
projections, RoPE, full-sequence attention, and a partial output
projection (its heads' rows of w_out).  Host sums the two partial
outputs per batch.

All matmul operands are bf16 (fp32 weights can't use the PE background
weight buffer, so their LDWEIGHTS serialize with the matmuls and the
HAM clock never warms).  x itself ships bf16 (halves the input DMA).
RMS-norm runs on ACT (Square + Sqrt).  Softmax exp runs on ACT (spline
exp), with an optional per-kt offload to DVE (single-op Schraudolph
bit-trick: tensor_scalar to int16, bitcast bf16); offloaded tiles'
attnV matmuls flush out-of-order (PSUM accumulation order is free).
The denominator comes from a ones column appended to V (attn@V row
64).  Logit matmuls contract 64 partitions, so the two heads of a pair
are issued adjacently at base partitions 0/64 to run concurrently in
separate PE row-groups.  Output projection is emitted per 512-query
chunk right after the chunk's two head-pair blocks, so the y DMA
overlaps attention.
"""

import math
import os
from contextlib import ExitStack

import numpy as np

import concourse.bass as bass
import concourse.tile as tile
from concourse import bacc, mybir

B, N, DIM = 4, 2048, 512
H, D = 8, 64
ROPE_THETA = 10000.0
NCORES = 8
SCALE = D ** -0.5

F32 = mybir.dt.float32
BF16 = mybir.dt.bfloat16
I16 = mybir.dt.int16

# Schraudolph exp in bf16-bits domain: e ~= bitcast_bf16(int16(A2*x + B2))
A2 = 128.0 / math.log(2.0)
B2 = 16250.5

# kt indices whose exp tiles run on DVE instead of ACT (tunable).  The
# DVE path is the single-op Schraudolph bit-trick; it is slower per tile
# than ACT spline exp (~1.5us vs ~1.0us) but runs on an otherwise-idle
# engine, so a minority share goes to DVE.
_dve_env = os.environ.get("KERNEL_DVE_KTS")
DVE_KTS = (set(int(t) for t in _dve_env.split(",") if t != "")
           if _dve_env is not None else {2, 5, 8, 11, 14})
EXP2OP = bool(os.environ.get("KERNEL_EXP2OP"))


def build_program():
    nc = bacc.Bacc("TRN2", target_bir_lowering=False, debug=False)

    xT = nc.dram_tensor("xT", [DIM, N], BF16, kind="ExternalInput").ap()
    wqk = nc.dram_tensor("wqk", [DIM, 512], BF16, kind="ExternalInput").ap()
    wv = nc.dram_tensor("wv", [DIM, 256], BF16, kind="ExternalInput").ap()
    wo = nc.dram_tensor("wo", [256, DIM], BF16, kind="ExternalInput").ap()
    cos2 = nc.dram_tensor("cos2", [128, N], BF16, kind="ExternalInput").ap()
    sinF2 = nc.dram_tensor("sinF2", [128, N], BF16, kind="ExternalInput").ap()
    yT = nc.dram_tensor("yT", [DIM, N], F32, kind="ExternalOutput").ap()

    DEBUG_TAPS = bool(os.environ.get("KERNEL_DEBUG_TAPS"))
    taps = {}
    if DEBUG_TAPS:
        for nm, shape, dt in [
            ("t_sinv", [128, N], F32),
            ("t_xn0", [128, N], BF16),
            ("t_qr0", [128, N], BF16),
            ("t_kr0", [128, N], BF16),
            ("t_v0", [128, 260], BF16),
            ("t_od00", [128, 1024], BF16),
        ]:
            taps[nm] = nc.dram_tensor(nm, shape, dt, kind="ExternalOutput").ap()

    AF = mybir.ActivationFunctionType

    with tile.TileContext(nc) as tc:
        with ExitStack() as ctx:
            persist = ctx.enter_context(tc.tile_pool(name="persist", bufs=1))
            work = ctx.enter_context(tc.tile_pool(name="work", bufs=3))
            rwork = ctx.enter_context(tc.tile_pool(name="rwork", bufs=2))
            ps_s = ctx.enter_context(tc.tile_pool(name="ps_s", bufs=2, space="PSUM"))
            ps_v = ctx.enter_context(tc.tile_pool(name="ps_v", bufs=1, space="PSUM"))
            ps_o = ctx.enter_context(tc.tile_pool(name="ps_o", bufs=2, space="PSUM"))
            exps = ctx.enter_context(tc.tile_pool(name="exps", bufs=6))
            aff = ctx.enter_context(tc.tile_pool(name="aff", bufs=2))
            rcp = ctx.enter_context(tc.tile_pool(name="rcp", bufs=2))
            ysb = ctx.enter_context(tc.tile_pool(name="ysb", bufs=3))

            # ---- inputs: few large DMA descriptors (each trigger costs
            # ~700ns of queue issue time).  sync queue: x halves + wqk +
            # rope tables (the early critical path); gpsimd queue: memsets
            # then wv/wo (needed later); scalar queue stays empty so the
            # first Squares issue immediately ----
            xt = [persist.tile([128, N], BF16, tag=f"xt{i}", name=f"xt{i}")
                  for i in range(4)]
            for i in range(4):
                nc.sync.dma_start(xt[i][:, 0:1024],
                                  xT[i * 128:(i + 1) * 128, 0:1024])
            ones128 = persist.tile([128, 128], BF16, tag="ones128",
                                   name="ones128")
            nc.gpsimd.memset(ones128[:], 1.0)
            wqk_t = []
            for i in range(4):
                t = persist.tile([128, 512], BF16, tag=f"wqk{i}", name=f"wqk{i}")
                nc.sync.dma_start(t[:], wqk[i * 128:(i + 1) * 128, :])
                wqk_t.append(t)
            for i in range(4):
                nc.sync.dma_start(xt[i][:, 1024:2048],
                                  xT[i * 128:(i + 1) * 128, 1024:2048])
            cos_t = persist.tile([128, N], BF16, tag="cos", name="cos")
            nc.sync.dma_start(cos_t[:], cos2)
            sin_t = persist.tile([128, N], BF16, tag="sin", name="sin")
            nc.sync.dma_start(sin_t[:], sinF2)
            wv_t = []
            for i in range(4):
                t = persist.tile([128, 256], BF16, tag=f"wv{i}", name=f"wv{i}")
                nc.gpsimd.dma_start(t[:], wv[i * 128:(i + 1) * 128, :])
                wv_t.append(t)
            wo_t = []
            for p in range(2):
                t = persist.tile([128, 512], BF16, tag=f"wo{p}", name=f"wo{p}")
                nc.gpsimd.dma_start(t[:], wo[p * 128:(p + 1) * 128, :])
                wo_t.append(t)
            # V tiles: head j at cols [65j, 65j+64), col 65j+64 == 1.0;
            # memset the whole tile, V parts get overwritten by the evac copy
            v_sb = []
            for tt in range(16):
                t = persist.tile([128, 260], BF16, tag=f"v{tt}", name=f"v{tt}")
                nc.gpsimd.memset(t[:], 1.0)
                v_sb.append(t)

            # ---- phase A: rmsnorm (ACT squares + sqrt), xn bf16,
            # pipelined per 512-column chunk ----
            sinv = persist.tile([128, N], F32, tag="sinv", name="sinv")
            xn = [persist.tile([128, N], BF16, tag=f"xn{i}", name=f"xn{i}")
                  for i in range(4)]
            for c in range(4):
                cs = slice(c * 512, (c + 1) * 512)
                ss = ps_s.tile([128, 512], F32, tag="sc", name="ss")
                for i in range(4):
                    xsq = work.tile([128, 512], BF16, tag="xsq", name="xsq",
                                    bufs=3)
                    if i < 2:
                        nc.scalar.activation(xsq[:], xt[i][:, cs], AF.Square)
                    else:
                        nc.vector.tensor_mul(xsq[:], xt[i][:, cs],
                                             xt[i][:, cs])
                    nc.tensor.matmul(ss[:], ones128[:], xsq[:],
                                     start=(i == 0), stop=(i == 3))
                # snorm = sqrt(sumsq/512)  ->  1/snorm = sqrt(512)/||x||
                sn = work.tile([128, 512], F32, tag="snorm", name="snorm")
                nc.scalar.activation(sn[:], ss[:], AF.Sqrt, scale=1.0 / DIM)
                nc.vector.reciprocal_approx_fast(sinv[:, cs], sn[:])
                for i in range(4):
                    nc.vector.tensor_mul(xn[i][:, cs], xt[i][:, cs],
                                         sinv[:, cs])

            # ---- Q/K projection + RoPE (whole m-tile at FD=2048) ----
            # wqk columns: [q h0..h3 | k h0..h3]; m=0: q pair0, m=1: q
            # pair1, m=2: k pair0, m=3: k pair1.  Head d-dims host-permuted
            # to [evens | odds] so the RoPE pair-swap is 32-row block moves.
            qk_dest = []
            for name in ["qr0", "qr1", "kr0", "kr1"]:
                t = persist.tile([128, N], BF16, tag=name, name=name)
                qk_dest.append(t)

            def make_rope_stages(m, cast_on_act=True):
                """Staged RoPE for m-tile m: returns (chunk_fns, tail_fns).
                chunk_fns (4): QK-proj matmuls + PSUM evac per 512-col chunk.
                tail_fns (7): pair-swaps, cos/sin muls, final add.  Each fn
                is a closure so the pieces can be interleaved into an
                ACT-bound attention block's emission stream."""
                ms = slice(m * 128, (m + 1) * 128)
                qkb = rwork.tile([128, N], BF16, tag="qkb", name="qkb")
                rotu = rwork.tile([128, N], BF16, tag="rotu", name="rotu")

                def chunk(c):
                    cs = slice(c * 512, (c + 1) * 512)
                    qk = ps_s.tile([128, 512], F32, tag="sc", name="qkps")
                    for i in range(4):
                        nc.tensor.matmul(qk[:], wqk_t[i][:, ms],
                                         xn[i][:, cs],
                                         start=(i == 0), stop=(i == 3))
                    if cast_on_act:
                        nc.scalar.copy(qkb[:, cs], qk[:])
                    else:
                        nc.vector.tensor_copy(qkb[:, cs], qk[:])

                # pair-swap as int32 moves (halves the DVE element count)
                qkb_i = qkb[:].bitcast(mybir.dt.int32)
                rotu_i = rotu[:].bitcast(mybir.dt.int32)

                def swap(h0, up):
                    if up:
                        nc.vector.tensor_copy(rotu_i[h0:h0 + 32, :],
                                              qkb_i[h0 + 32:h0 + 64, :])
                    else:
                        nc.vector.tensor_copy(rotu_i[h0 + 32:h0 + 64, :],
                                              qkb_i[h0:h0 + 32, :])

                dst = qk_dest[m]
                chunk_fns = [lambda c=c: chunk(c) for c in range(4)]
                tail_fns = (
                    [lambda h0=h0, up=up: swap(h0, up)
                     for h0 in (0, 64) for up in (True, False)]
                    + [lambda: nc.vector.tensor_mul(dst[:], qkb[:], cos_t[:]),
                       lambda: nc.vector.tensor_mul(rotu[:], rotu[:],
                                                    sin_t[:]),
                       lambda: nc.vector.tensor_add(dst[:], dst[:], rotu[:])])
                return chunk_fns, tail_fns

            def emit_rope(m, cast_on_act=True):
                chunk_fns, tail_fns = make_rope_stages(m, cast_on_act)
                for f in chunk_fns + tail_fns:
                    f()

            # ---- V projection (token-major), ones column per head ----
            # v_sb layout: head j at cols [65j, 65j+64), col 65j+64 == 1.0
            def make_v_fns():
                def one(tt):
                    vp = ps_s.tile([128, 512], F32, tag="sc", name="vps")
                    ts = slice(tt * 128, (tt + 1) * 128)
                    for i in range(4):
                        nc.tensor.matmul(vp[:, 0:256], xn[i][:, ts], wv_t[i][:],
                                         start=(i == 0), stop=(i == 3))
                    dst = v_sb[tt][:].rearrange("p (h c) -> p h c", h=4)[:, :, 0:64]
                    src = vp[:, 0:256].rearrange("p (h c) -> p h c", h=4)
                    nc.scalar.copy(dst, src)
                return [lambda tt=tt: one(tt) for tt in range(16)]

            # ---- attention for one (query-quarter, head-pair) ----
            outd = [[None, None], [None, None]]

            def emit_attention(qq, hp, filler=None, dve_kts=None):
                # query-quarter block (512 queries, head pair hp): each kt
                # gets ONE [128,1024] s tile holding both heads' logits
                # (j0 cols 0:512 -> bank 0, j1 cols 512:1024 -> bank 1).
                # The two 64-contract logit matmuls run concurrently in
                # different PE row groups; one exp covers both heads.
                # logits(kt+1) are emitted before attnV(kt) so the PE
                # pipelines past the exp latency.
                qr, kr = qk_dest[hp], qk_dest[2 + hp]
                qh, qsub = qq // 2, qq % 2
                qs = slice(qq * 512, (qq + 1) * 512)
                od = outd[hp][qh]
                if od is None:
                    od = persist.tile([128, 1024], BF16, tag=f"od{hp}{qh}",
                                      name=f"od{hp}{qh}")
                    outd[hp][qh] = od
                o_ps = [ps_o.tile([65, 512], F32, tag="o", name="o")
                        for _ in range(2)]
                ndone = [0]

                def emit_attnv(kt, e):
                    for j in range(2):
                        h = 2 * hp + j
                        nc.tensor.matmul(
                            o_ps[j][:],
                            v_sb[kt][:, 65 * h:65 * h + 65],
                            e[:, j * 512:(j + 1) * 512],
                            start=(ndone[0] == 0), stop=(ndone[0] == 15),
                            skip_group_check=True)
                    ndone[0] += 1

                # pending attnVs: ACT-exp kts flush at skew 1; DVE-exp kts
                # at skew 2 (the DVE exp has higher latency); accumulation
                # order is free so later-ready tiles flush out of order
                if dve_kts is None:
                    dve_kts = DVE_KTS
                pend = []
                for kt in range(16):
                    ks = slice(kt * 128, (kt + 1) * 128)
                    if kt in dve_kts:
                        # DVE-destined logits go to a dedicated 1-slot PSUM
                        # ring so DVE queue latency never stalls the ACT/PE
                        # ring.  DVE kts must be spaced >= 3 apart.
                        s_ps = ps_v.tile([128, 1024], F32, tag="scv",
                                         name="scv")
                    else:
                        s_ps = ps_s.tile([128, 1024], F32, tag="sc",
                                         name="sc")
                    for j in range(2):
                        js = slice(j * 64, (j + 1) * 64)
                        nc.tensor.matmul(
                            s_ps[:, j * 512:(j + 1) * 512],
                            kr[js, ks], qr[js, qs],
                            start=True, stop=True)
                    if kt in dve_kts:
                        # single-op Schraudolph bit-trick exp on DVE
                        ei = exps.tile([128, 1024], I16, tag="e", name="e")
                        nc.vector.tensor_scalar(
                            ei[:], s_ps[:], SCALE * A2, B2,
                            mybir.AluOpType.mult, mybir.AluOpType.add)
                        e = ei[:].bitcast(BF16)
                        skew = 2
                    else:
                        eb = exps.tile([128, 1024], BF16, tag="e", name="e")
                        nc.scalar.activation(eb[:], s_ps[:], AF.Exp,
                                             scale=SCALE)
                        e = eb[:]
                        skew = 1
                    pend.append((kt, e, kt + skew))
                    for item in [p for p in pend if p[2] <= kt]:
                        pend.remove(item)
                        emit_attnv(item[0], item[1])
                    if filler is not None:
                        filler(kt)
                for k0, e0, _ in pend:
                    emit_attnv(k0, e0)

                # normalize: stage the ones row to SBUF (custom-DVE recip
                # doesn't honor PSUM base_partition), reciprocal, gpsimd
                # broadcast, then one fused evac-multiply per head
                ods = slice(qsub * 512, (qsub + 1) * 512)
                for j in range(2):
                    js = slice(j * 64, (j + 1) * 64)
                    dcopy = rcp.tile([1, 512], F32, tag="dcopy", name="dcopy",
                                     bufs=2)
                    nc.vector.tensor_copy(dcopy[:], o_ps[j][64:65, :])
                    rrow = rcp.tile([1, 512], F32, tag="rrow", name="rrow",
                                    bufs=2)
                    nc.vector.reciprocal_approx_fast(rrow[:], dcopy[:])
                    rfull = rcp.tile([64, 512], F32, tag="rfull", name="rfull")
                    nc.gpsimd.partition_broadcast(rfull[:], rrow[:])
                    nc.vector.tensor_mul(od[js, ods], o_ps[j][0:64, :],
                                         rfull[:])

            # ---- output projection for one query-quarter (partial w_out) ----
            def emit_proj_q(qq):
                qh, qsub = qq // 2, qq % 2
                ods = slice(qsub * 512, (qsub + 1) * 512)
                for om in range(4):
                    oms = slice(om * 128, (om + 1) * 128)
                    yp = ps_s.tile([128, 512], F32, tag="sc", name="yp")
                    for p in range(2):
                        nc.tensor.matmul(
                            yp[:], wo_t[p][:, oms], outd[p][qh][:, ods],
                            start=(p == 0), stop=(p == 1))
                    yo = ysb.tile([128, 512], F32, tag="y", name="y")
                    nc.vector.tensor_copy(yo[:], yp[:])
                    nc.scalar.dma_start(
                        yT[oms, qq * 512:(qq + 1) * 512], yo[:])

            # ---- emission order: K/V/Q for pair 0 first so attention
            # starts early.  Pair-1 rope is CHUNK-INTERLEAVED into the
            # first three (ACT-bound) attention blocks: its matmuls and
            # DVE ops fill the PE/DVE gaps while ACT grinds exps.  All
            # four hp=0 blocks run first (they only need pair-0 rope);
            # output projection per query-quarter, emitted one block late
            # so its matmuls fill PE gaps too ----
            def filler_from(schedule):
                # schedule: {kt: [fn, ...]}
                def filler(kt):
                    for fn in schedule.get(kt, ()):
                        fn()
                return filler

            emit_rope(2)      # kr0
            qr0_chunks, qr0_tails = make_rope_stages(0)
            for f in qr0_chunks:
                f()
            v_fns = make_v_fns()
            for f in v_fns[0:4]:   # v_sb 0-3 must precede A00's attnv kt0-3
                f()
            for f in qr0_tails:
                f()
            # (stage-tile allocation must follow the pair-0 ropes: the
            # qkb/rotu rings have 2 slots, and ring order is emission order)
            kr1_chunks, kr1_tails = make_rope_stages(3, cast_on_act=False)
            qr1_chunks, qr1_tails = make_rope_stages(1, cast_on_act=False)
            # block A00: the remaining 12 V tiles ride the exp gaps
            # (attnv kt needs v_sb[kt], so group c must land by kt=4c)
            emit_attention(0, 0, filler_from(
                {0: [v_fns[4]], 1: [v_fns[5]], 2: [v_fns[6], v_fns[7]],
                 4: [v_fns[8]], 5: [v_fns[9]], 6: [v_fns[10], v_fns[11]],
                 8: [v_fns[12]], 9: [v_fns[13]],
                 10: [v_fns[14], v_fns[15]]}),
                dve_kts={2, 5, 8, 11, 14})
            # block A10: kr1 + qr1 qk-proj matmul chunks
            emit_attention(1, 0, filler_from(
                {1: [kr1_chunks[0]], 3: [kr1_chunks[1]],
                 5: [kr1_chunks[2]], 7: [kr1_chunks[3]],
                 9: [qr1_chunks[0]], 11: [qr1_chunks[1]],
                 13: [qr1_chunks[2]], 15: [qr1_chunks[3]]}),
                dve_kts={5, 11})
            # block A20: kr1 swap/mul tail
            emit_attention(2, 0, filler_from(
                {1: [kr1_tails[0]], 3: [kr1_tails[1]],
                 5: [kr1_tails[2]], 7: [kr1_tails[3]],
                 9: [kr1_tails[4]], 11: [kr1_tails[5]],
                 13: [kr1_tails[6]]}), dve_kts={5, 11})
            # block A30: qr1 swap/mul tail
            emit_attention(3, 0, filler_from(
                {1: [qr1_tails[0]], 3: [qr1_tails[1]],
                 5: [qr1_tails[2]], 7: [qr1_tails[3]],
                 9: [qr1_tails[4]], 11: [qr1_tails[5]],
                 13: [qr1_tails[6]]}), dve_kts={5, 11})
            emit_attention(0, 1)

            def proj_chunks(qq):
                qh, qsub = qq // 2, qq % 2
                ods = slice(qsub * 512, (qsub + 1) * 512)

                def one(om):
                    oms = slice(om * 128, (om + 1) * 128)
                    yp = ps_s.tile([128, 512], F32, tag="sc", name="yp")
                    for p in range(2):
                        nc.tensor.matmul(
                            yp[:], wo_t[p][:, oms], outd[p][qh][:, ods],
                            start=(p == 0), stop=(p == 1))
                    yo = ysb.tile([128, 512], F32, tag="y", name="y")
                    nc.vector.tensor_copy(yo[:], yp[:])
                    nc.scalar.dma_start(
                        yT[oms, qq * 512:(qq + 1) * 512], yo[:])
                return [lambda om=om: one(om) for om in range(4)]

            p0 = proj_chunks(0)
            # A11: P0 rides (deps A00/A01 done)
            emit_attention(1, 1, filler_from(
                {1: [p0[0]], 3: [p0[1]], 5: [p0[2]], 7: [p0[3]]}),
                dve_kts={2, 6, 10, 14})
            p1 = proj_chunks(1)
            emit_attention(2, 1, filler_from(
                {1: [p1[0]], 3: [p1[1]], 5: [p1[2]], 7: [p1[3]]}),
                dve_kts={2, 6, 10, 14})
            p2 = proj_chunks(2)
            emit_attention(3, 1, filler_from(
                {1: [p2[0]], 3: [p2[1]], 5: [p2[2]], 7: [p2[3]]}),
                dve_kts={2, 6, 10, 14})
            emit_proj_q(3)

            if DEBUG_TAPS:
                nc.sync.dma_start(taps["t_sinv"], sinv[:])
                nc.sync.dma_start(taps["t_xn0"], xn[0][:])
                nc.sync.dma_start(taps["t_qr0"], qk_dest[0][:])
                nc.sync.dma_start(taps["t_kr0"], qk_dest[2][:])
                nc.sync.dma_start(taps["t_v0"], v_sb[0][:])
                nc.sync.dma_start(taps["t_od00"], outd[0][0][:])

    nc.compile()
    return nc


def rope_tables():
    """cos / sign-folded sin tables in permuted ([evens | odds]) row order,
    stacked for two 64-row head slots.

    Device row r in [0, 32): holds d-dim 2r (even slot, rot sign -1);
    row r in [32, 64): d-dim 2(r-32)+1 (odd slot, rot sign +1).  Both use
    frequency index r % 32.
    """
    inv_freq = (1.0 / (ROPE_THETA ** (np.arange(0, D, 2, dtype=np.float32) / D)))
    freqs = np.arange(N, dtype=np.float32)[:, None] * inv_freq[None, :]  # [N, 32]
    cos = np.cos(freqs).T.astype(np.float32)  # [32, N]
    sin = np.sin(freqs).T.astype(np.float32)  # [32, N]
    cos64 = np.concatenate([cos, cos], axis=0)  # [64, N]
    sinF64 = np.concatenate([-sin, sin], axis=0)
    cos2 = np.concatenate([cos64, cos64], axis=0)  # [128, N]
    sinF2 = np.concatenate([sinF64, sinF64], axis=0)
    return np.ascontiguousarray(cos2), np.ascontiguousarray(sinF2)


_PERM64 = np.concatenate([np.arange(0, D, 2), np.arange(1, D, 2)])


def _permute_heads(w):
    """Permute each head's 64 columns of w [512, 256] to [evens | odds]."""
    w = w.reshape(DIM, 4, D)[:, :, _PERM64]
    return w.reshape(DIM, 256)


def _bf16():
    import ml_dtypes
    return ml_dtypes.bfloat16


def make_in_maps(x, gamma, w_qkv, w_out):
    bf = _bf16()
    cos2, sinF2 = rope_tables()
    wg = (gamma[:, None] * w_qkv).astype(np.float32)  # fold gamma
    in_maps = []
    for c in range(NCORES):
        b, g = c // 2, c % 2
        hs = slice(g * 256, (g + 1) * 256)
        wqk_c = np.concatenate([_permute_heads(wg[:, 0:512][:, hs]),
                                _permute_heads(wg[:, 512:1024][:, hs])],
                               axis=1)
        wv_c = wg[:, 1024:1536][:, hs]
        wo_c = w_out[hs, :]
        in_maps.append({
            "xT": np.ascontiguousarray(x[b].T).astype(bf),
            "wqk": np.ascontiguousarray(wqk_c).astype(bf),
            "wv": np.ascontiguousarray(wv_c).astype(bf),
            "wo": np.ascontiguousarray(wo_c).astype(bf),
            "cos2": cos2.astype(bf),
            "sinF2": sinF2.astype(bf),
        })
    return in_maps


_NC_CACHE = None


def _get_program():
    global _NC_CACHE
    if _NC_CACHE is None:
        _NC_CACHE = build_program()
    return _NC_CACHE


def run_cores(inputs, trace=False):
    """Run the SPMD kernel on 8 cores; returns (full_output, results)."""
    from concourse.bass_utils import run_bass_kernel_spmd

    nc = _get_program()
    in_maps = make_in_maps(inputs["x"], inputs["gamma"],
                           inputs["w_qkv"], inputs["w_out"])
    kwargs = {}
    if trace:
        _install_ntff_hook()
        kwargs = dict(trace=True, trace_cores=list(range(NCORES)))
    res = run_bass_kernel_spmd(nc, in_maps, core_ids=list(range(NCORES)),
                               **kwargs)
    out = np.empty((B, N, DIM), dtype=np.float32)
    for b in range(B):
        yTv = res.results[2 * b]["yT"] + res.results[2 * b + 1]["yT"]
        out[b] = yTv.T
    return out, res


def _install_ntff_hook():
    """Register the axon NTFF profiling hook (missing antenv.axon_hooks)."""
    import sys
    import types

    if "antenv.axon_hooks" in sys.modules:
        return
    try:
        import trn_agent_boot.trn_boot as tb
        import concourse.bass_utils as bu

        mod = types.ModuleType("antenv.axon_hooks")
        hook = tb._ntff_profile_via_ctypes("/opt/axon/libaxon_pjrt.so")
        mod.get_axon_ntff_profile_hook = lambda: hook
        sys.modules["antenv.axon_hooks"] = mod
        bu.upload_artifacts = lambda tmpdir: "local://" + tmpdir
    except Exception:
        pass


def kernel(**inputs):
    out, _ = run_cores(inputs, trace=bool(os.environ.get("KERNEL_TRACE")))
    return out
